# revision 2
# baseline (speedup 1.0000x reference)
"""FP8 blockwise QDQ linear (LumenLinear) on 8 TRN2 NeuronCores.

out = dequant(Q_fp8(x)) @ dequant(Q_fp8(W)).T + bias
  x [8192, 4096] f32, blockwise (1x128) act quant along K
  W [11008, 4096] f32, blockwise (128x128) weight quant
  out [8192, 11008] f32

Strategy: tensor-parallel shard W along out_features across 8 cores
(pad 11008 -> 11264 = 8*1408), replicate x. Per core, on device:
  - exact e4m3fn-grid QDQ using TRN fp8e4 with scale = max(amax,eps)/224
    (factor-2 rescale maps the OCP e4m3fn grid onto TRN's +-240 e4m3 grid)
  - dequantized operands stored fp16, x transposed K-major via DMA xbar
  - fp16 matmuls accumulate K=4096 into PSUM f32; evict via ScalarE
    (bias-zero fast path) or VectorE add (general bias)
"""

import numpy as np
from contextlib import ExitStack

P = 128
M, K, N_FULL = 8192, 4096, 11008
NCORES = 8
N_PAD = 11264            # 88 blocks of 128
NC_ = N_PAD // NCORES    # 1408 per core
KT = K // P              # 32 k-tiles
MT = M // P              # 64 m-tiles
NB = NC_ // P            # 11 n-blocks per core
CHUNKS = [(0, 512), (512, 512), (1024, 384)]  # psum chunks of NC_

_CACHE = {}
LAST_RES = None


def _build(with_bias):
    import concourse.bass as bass
    import concourse.mybir as mybir
    import concourse.tile as tile
    import concourse.bass_isa as bass_isa
    from concourse import bacc

    FP32 = mybir.dt.float32
    FP16 = mybir.dt.float16
    FP8 = mybir.dt.float8e4

    nc = bacc.Bacc("TRN2", target_bir_lowering=False, debug=False,
                   num_devices=NCORES)
    x_d = nc.dram_tensor("x", [M, K], FP32, kind="ExternalInput").ap()
    wT_d = nc.dram_tensor("wT", [K, NC_], FP32, kind="ExternalInput").ap()
    bias_h = nc.dram_tensor("bias", [1, NC_], FP32, kind="ExternalInput")
    out_d = nc.dram_tensor("out", [M, NC_], FP32, kind="ExternalOutput").ap()

    with tile.TileContext(nc) as tc, ExitStack() as ctx:
        singles = ctx.enter_context(tc.tile_pool(name="singles", bufs=1))

        bias_bc = None
        if with_bias:
            bias_bc = singles.tile([P, NC_], FP32)
            bias_src = bass.AP(tensor=bias_h, offset=0, ap=[[0, P], [1, NC_]])
            nc.gpsimd.dma_start(out=bias_bc[:], in_=bias_src)

        # resident dequantized weight, [128 k, KT, NC_] fp16
        wdq = singles.tile([P, KT, NC_], FP16)

        # ---- Phase W: quantize weight k-tile by k-tile
        with tc.tile_pool(name="wpool", bufs=3) as wpool, \
             tc.tile_pool(name="wsc", bufs=3) as wsc:
            for kt in range(KT):
                wld = wpool.tile([P, NC_], FP32, tag="wld")
                nc.sync.dma_start(wld[:], wT_d[kt * P:(kt + 1) * P, :])
                wam = wsc.tile([P, NB], FP32, tag="wam")
                nc.vector.tensor_reduce(
                    wam[:], wld[:].rearrange("p (nb b) -> p nb b", b=P),
                    axis=mybir.AxisListType.X, op=mybir.AluOpType.max,
                    apply_absolute_value=True)
                wamr = wsc.tile([P, NB], FP32, tag="wamr")
                nc.gpsimd.partition_all_reduce(
                    wamr[:], wam[:], channels=P, reduce_op=bass_isa.ReduceOp.max)
                wt_ = wsc.tile([P, NB], FP32, tag="wt_")
                nc.vector.tensor_scalar_max(wt_[:], wamr[:], 1e-12)
                winv = wsc.tile([P, NB], FP32, tag="winv")
                nc.vector.reciprocal(winv[:], wt_[:])
                nc.vector.tensor_scalar_mul(winv[:], winv[:], 224.0)
                wd = wsc.tile([P, NB], FP32, tag="wd")
                nc.vector.tensor_scalar_mul(wd[:], wt_[:], 1.0 / 224.0)

                wq8 = wpool.tile([P, NC_], FP8, tag="wq8")
                for nb in range(NB):
                    nsl = slice(nb * P, (nb + 1) * P)
                    nc.scalar.mul(wq8[:, nsl], wld[:, nsl], winv[:, nb:nb + 1])
                    nc.scalar.mul(wdq[:, kt, nsl], wq8[:, nsl], wd[:, nb:nb + 1])

        # ---- Phase X: per m-tile quantize, transpose, matmul
        xpool = ctx.enter_context(tc.tile_pool(name="xpool", bufs=2))
        xq = ctx.enter_context(tc.tile_pool(name="xq", bufs=3))
        xsc = ctx.enter_context(tc.tile_pool(name="xsc", bufs=3))
        opool = ctx.enter_context(tc.tile_pool(name="opool", bufs=2))
        psum = ctx.enter_context(tc.tile_pool(name="psum", bufs=6, space="PSUM"))

        for mt in range(MT):
            xld = xpool.tile([P, K], FP32, tag="xld")
            nc.sync.dma_start(xld[:], x_d[mt * P:(mt + 1) * P, :])
            xam = xsc.tile([P, KT], FP32, tag="xam")
            nc.vector.tensor_reduce(
                xam[:], xld[:].rearrange("p (t b) -> p t b", b=P),
                axis=mybir.AxisListType.X, op=mybir.AluOpType.max,
                apply_absolute_value=True)
            xt_ = xsc.tile([P, KT], FP32, tag="xt_")
            nc.vector.tensor_scalar_max(xt_[:], xam[:], 1e-12)
            xinv = xsc.tile([P, KT], FP32, tag="xinv")
            nc.vector.reciprocal(xinv[:], xt_[:])
            nc.vector.tensor_scalar_mul(xinv[:], xinv[:], 224.0)
            xd = xsc.tile([P, KT], FP32, tag="xd")
            nc.vector.tensor_scalar_mul(xd[:], xt_[:], 1.0 / 224.0)

            q8 = xq.tile([P, K], FP8, tag="q8")
            xinv_bc = xinv[:].rearrange("p (t o) -> p t o", o=1).broadcast_to([P, KT, P])
            nc.vector.tensor_tensor(
                out=q8[:].rearrange("p (t b) -> p t b", b=P),
                in0=xld[:].rearrange("p (t b) -> p t b", b=P),
                in1=xinv_bc, op=mybir.AluOpType.mult)
            xdq = xq.tile([P, K], FP16, tag="xdq")
            xd_bc = xd[:].rearrange("p (t o) -> p t o", o=1).broadcast_to([P, KT, P])
            nc.vector.tensor_tensor(
                out=xdq[:].rearrange("p (t b) -> p t b", b=P),
                in0=q8[:].rearrange("p (t b) -> p t b", b=P),
                in1=xd_bc, op=mybir.AluOpType.mult)

            xT = xq.tile([P, KT, P], FP16, tag="xT")
            nc.sync.dma_start_transpose(xT[:], xdq[:])

            osb = opool.tile([P, NC_], FP32, tag="osb")
            for (off, cw) in CHUNKS:
                ps = psum.tile([P, cw], FP32, tag="ps")
                for kt in range(KT):
                    nc.tensor.matmul(
                        ps[:], xT[:, kt, :], wdq[:, kt, off:off + cw],
                        start=(kt == 0), stop=(kt == KT - 1))
                if with_bias:
                    nc.vector.tensor_tensor(
                        out=osb[:, off:off + cw], in0=ps[:],
                        in1=bias_bc[:, off:off + cw], op=mybir.AluOpType.add)
                else:
                    nc.scalar.copy(osb[:, off:off + cw], ps[:])
            nc.sync.dma_start(out_d[mt * P:(mt + 1) * P, :], osb[:])

    nc.compile()
    return nc


def kernel(input, weight, bias):
    global LAST_RES
    from concourse.bass_utils import run_bass_kernel_spmd

    with_bias = bool(np.any(np.asarray(bias)))
    key = ("nc", with_bias)
    if key not in _CACHE:
        _CACHE[key] = _build(with_bias)
    nc = _CACHE[key]

    x = np.ascontiguousarray(input, dtype=np.float32)
    wpad = np.zeros((N_PAD, K), dtype=np.float32)
    wpad[:N_FULL] = weight
    wT = wpad.T  # [K, N_PAD] view
    bpad = np.zeros((N_PAD,), dtype=np.float32)
    bpad[:N_FULL] = bias

    in_maps = []
    for c in range(NCORES):
        sl = slice(c * NC_, (c + 1) * NC_)
        in_maps.append({
            "x": x,
            "wT": np.ascontiguousarray(wT[:, sl]),
            "bias": np.ascontiguousarray(bpad[sl]).reshape(1, NC_),
        })

    res = run_bass_kernel_spmd(nc, in_maps, list(range(NCORES)))
    LAST_RES = res
    out = np.concatenate([res.results[c]["out"] for c in range(NCORES)], axis=1)
    return np.ascontiguousarray(out[:, :N_FULL])


# revision 4
# speedup vs baseline: 1.0880x; 1.0880x over previous
"""FP8 blockwise QDQ linear (LumenLinear) on 8 TRN2 NeuronCores.

out = dequant(Q_fp8(x)) @ dequant(Q_fp8(W)).T + bias
  x [8192, 4096] f32, blockwise (1x128) act quant along K
  W [11008, 4096] f32, blockwise (128x128) weight quant
  out [8192, 11008] f32

Strategy: tensor-parallel shard W along out_features across 8 cores
(pad 11008 -> 11264 = 8*1408), replicate x. Per core, on device:
  - exact e4m3fn-grid QDQ using TRN fp8e4 with scale = max(amax,eps)/224
    (factor-2 rescale maps the OCP e4m3fn grid onto TRN's +-240 e4m3 grid)
  - dequantized operands stored fp16, x transposed K-major via DMA xbar
  - fp16 matmuls accumulate K=4096 into PSUM f32; evict via ScalarE
    (bias-zero fast path) or VectorE add (general bias)
"""

import numpy as np
from contextlib import ExitStack

P = 128
M, K, N_FULL = 8192, 4096, 11008
NCORES = 8
N_PAD = 11264            # 88 blocks of 128
NC_ = N_PAD // NCORES    # 1408 per core
KT = K // P              # 32 k-tiles
MT = M // P              # 64 m-tiles
NB = NC_ // P            # 11 n-blocks per core
CHUNKS = [(0, 512), (512, 512), (1024, 384)]  # psum chunks of NC_

_CACHE = {}
LAST_RES = None


def _build(with_bias):
    import concourse.bass as bass
    import concourse.mybir as mybir
    import concourse.tile as tile
    import concourse.bass_isa as bass_isa
    from concourse import bacc

    FP32 = mybir.dt.float32
    FP16 = mybir.dt.float16
    FP8 = mybir.dt.float8e4

    nc = bacc.Bacc("TRN2", target_bir_lowering=False, debug=False,
                   num_devices=NCORES)
    x_d = nc.dram_tensor("x", [M, K], FP32, kind="ExternalInput").ap()
    wT_d = nc.dram_tensor("wT", [K, NC_], FP32, kind="ExternalInput").ap()
    bias_h = nc.dram_tensor("bias", [1, NC_], FP32, kind="ExternalInput")
    out_d = nc.dram_tensor("out", [M, NC_], FP32, kind="ExternalOutput").ap()

    with tile.TileContext(nc) as tc, ExitStack() as ctx:
        singles = ctx.enter_context(tc.tile_pool(name="singles", bufs=1))

        bias_bc = None
        if with_bias:
            bias_bc = singles.tile([P, NC_], FP32)
            bias_src = bass.AP(tensor=bias_h, offset=0, ap=[[0, P], [1, NC_]])
            nc.gpsimd.dma_start(out=bias_bc[:], in_=bias_src)

        # resident dequantized weight, [128 k, KT, NC_] fp16
        wdq = singles.tile([P, KT, NC_], FP16)

        # ---- Phase W: quantize weight k-tile by k-tile
        with tc.tile_pool(name="wpool", bufs=3) as wpool, \
             tc.tile_pool(name="wsc", bufs=3) as wsc:
            for kt in range(KT):
                wld = wpool.tile([P, NC_], FP32, tag="wld")
                nc.sync.dma_start(wld[:], wT_d[kt * P:(kt + 1) * P, :])
                wam = wsc.tile([P, NB], FP32, tag="wam")
                nc.vector.tensor_reduce(
                    wam[:], wld[:].rearrange("p (nb b) -> p nb b", b=P),
                    axis=mybir.AxisListType.X, op=mybir.AluOpType.max,
                    apply_absolute_value=True)
                wamr = wsc.tile([P, NB], FP32, tag="wamr")
                nc.gpsimd.partition_all_reduce(
                    wamr[:], wam[:], channels=P, reduce_op=bass_isa.ReduceOp.max)
                wt_ = wsc.tile([P, NB], FP32, tag="wt_")
                nc.vector.tensor_scalar_max(wt_[:], wamr[:], 1e-12)
                winv = wsc.tile([P, NB], FP32, tag="winv")
                nc.vector.reciprocal(winv[:], wt_[:])
                nc.vector.tensor_scalar_mul(winv[:], winv[:], 224.0)
                wd = wsc.tile([P, NB], FP32, tag="wd")
                nc.vector.tensor_scalar_mul(wd[:], wt_[:], 1.0 / 224.0)

                wq8 = wpool.tile([P, NC_], FP8, tag="wq8")
                winv_bc = winv[:].rearrange("p (nb o) -> p nb o", o=1).broadcast_to([P, NB, P])
                nc.vector.tensor_tensor(
                    out=wq8[:].rearrange("p (nb b) -> p nb b", b=P),
                    in0=wld[:].rearrange("p (nb b) -> p nb b", b=P),
                    in1=winv_bc, op=mybir.AluOpType.mult)
                wd_bc = wd[:].rearrange("p (nb o) -> p nb o", o=1).broadcast_to([P, NB, P])
                nc.vector.tensor_tensor(
                    out=wdq[:, kt, :].rearrange("p (nb b) -> p nb b", b=P),
                    in0=wq8[:].rearrange("p (nb b) -> p nb b", b=P),
                    in1=wd_bc, op=mybir.AluOpType.mult)

        # ---- Phase X: per m-tile quantize, transpose, matmul
        xpool = ctx.enter_context(tc.tile_pool(name="xpool", bufs=2))
        xq = ctx.enter_context(tc.tile_pool(name="xq", bufs=3))
        xsc = ctx.enter_context(tc.tile_pool(name="xsc", bufs=3))
        opool = ctx.enter_context(tc.tile_pool(name="opool", bufs=2))
        psum = ctx.enter_context(tc.tile_pool(name="psum", bufs=8, space="PSUM"))

        for mt in range(MT):
            xld = xpool.tile([P, K], FP32, tag="xld")
            nc.sync.dma_start(xld[:], x_d[mt * P:(mt + 1) * P, :])
            xam = xsc.tile([P, KT], FP32, tag="xam")
            nc.vector.tensor_reduce(
                xam[:], xld[:].rearrange("p (t b) -> p t b", b=P),
                axis=mybir.AxisListType.X, op=mybir.AluOpType.max,
                apply_absolute_value=True)
            xt_ = xsc.tile([P, KT], FP32, tag="xt_")
            nc.vector.tensor_scalar_max(xt_[:], xam[:], 1e-12)
            xinv = xsc.tile([P, KT], FP32, tag="xinv")
            nc.vector.reciprocal(xinv[:], xt_[:])
            nc.vector.tensor_scalar_mul(xinv[:], xinv[:], 224.0)
            xd = xsc.tile([P, KT], FP32, tag="xd")
            nc.vector.tensor_scalar_mul(xd[:], xt_[:], 1.0 / 224.0)

            q8 = xq.tile([P, K], FP8, tag="q8")
            xinv_bc = xinv[:].rearrange("p (t o) -> p t o", o=1).broadcast_to([P, KT, P])
            nc.vector.tensor_tensor(
                out=q8[:].rearrange("p (t b) -> p t b", b=P),
                in0=xld[:].rearrange("p (t b) -> p t b", b=P),
                in1=xinv_bc, op=mybir.AluOpType.mult)
            xdq = xq.tile([P, K], FP16, tag="xdq")
            xd_bc = xd[:].rearrange("p (t o) -> p t o", o=1).broadcast_to([P, KT, P])
            nc.vector.tensor_tensor(
                out=xdq[:].rearrange("p (t b) -> p t b", b=P),
                in0=q8[:].rearrange("p (t b) -> p t b", b=P),
                in1=xd_bc, op=mybir.AluOpType.mult)

            xT = xq.tile([P, KT, P], FP16, tag="xT")
            nc.sync.dma_start_transpose(xT[:], xdq[:])

            osb = opool.tile([P, NC_], FP32, tag="osb")
            for (off, cw) in CHUNKS:
                ps = psum.tile([P, cw], FP32, tag="ps")
                for kt in range(KT):
                    nc.tensor.matmul(
                        ps[:], xT[:, kt, :], wdq[:, kt, off:off + cw],
                        start=(kt == 0), stop=(kt == KT - 1))
                if with_bias:
                    nc.vector.tensor_tensor(
                        out=osb[:, off:off + cw], in0=ps[:],
                        in1=bias_bc[:, off:off + cw], op=mybir.AluOpType.add)
                else:
                    nc.scalar.copy(osb[:, off:off + cw], ps[:])
            nc.sync.dma_start(out_d[mt * P:(mt + 1) * P, :], osb[:])

    nc.compile()
    return nc


def kernel(input, weight, bias):
    global LAST_RES
    from concourse.bass_utils import run_bass_kernel_spmd

    with_bias = bool(np.any(np.asarray(bias)))
    key = ("nc", with_bias)
    if key not in _CACHE:
        _CACHE[key] = _build(with_bias)
    nc = _CACHE[key]

    x = np.ascontiguousarray(input, dtype=np.float32)
    wpad = np.zeros((N_PAD, K), dtype=np.float32)
    wpad[:N_FULL] = weight
    wT = wpad.T  # [K, N_PAD] view
    bpad = np.zeros((N_PAD,), dtype=np.float32)
    bpad[:N_FULL] = bias

    in_maps = []
    for c in range(NCORES):
        sl = slice(c * NC_, (c + 1) * NC_)
        in_maps.append({
            "x": x,
            "wT": np.ascontiguousarray(wT[:, sl]),
            "bias": np.ascontiguousarray(bpad[sl]).reshape(1, NC_),
        })

    res = run_bass_kernel_spmd(nc, in_maps, list(range(NCORES)))
    LAST_RES = res
    out = np.concatenate([res.results[c]["out"] for c in range(NCORES)], axis=1)
    return np.ascontiguousarray(out[:, :N_FULL])
